# revision 1
# baseline (speedup 1.0000x reference)
"""AttentionBlock Trainium2 Bass kernel.

Data-parallel over batch: 16 batches / 8 cores = 2 per core. Each core runs
the full block (groupnorm x2, q/kv projections, 8-head attention, output
projection, residual) on its 2 batch elements.

Key design points:
- fp32r matmuls (full PE rate, ~1e-4 accuracy) for projections and QK;
  bf16 for softmax weights / vT (AV matmul).
- scoresT layout [s, t]: QK emits transposed scores so the AV matmul needs
  no transposes; softmax denominator comes from a ones-column appended to
  the vT stationary operand.
- softmax without max-subtraction (logits ~N(0,1), exp is safe in fp32).
- groupnorm stats via bn_stats + a block-diagonal membership matmul to
  reduce across partitions; rsqrt on DVE (quake seed + 2 Newton steps) to
  keep the ACT engine exclusively on exp.
- per-head q zero-padding trick: K=128 QK matmuls with the unused head's
  lanes zeroed, so no partition-sliced operands (avoids PE tiling modes).
- batch pipeline: next batch's loads + groupnorm + projections are emitted
  into the previous batch's attention phase.
"""
import os
import sys

sys.path.insert(0, "/opt/trn_rl_repo")

import numpy as np

import concourse.bacc as bacc
import concourse.bass as bass
import concourse.tile as tile
from concourse import mybir
from concourse.bass_utils import run_bass_kernel_spmd

F32 = mybir.dt.float32
F32R = mybir.dt.float32r
BF16 = mybir.dt.bfloat16
I32 = mybir.dt.int32
AF = mybir.ActivationFunctionType
OP = mybir.AluOpType

B, C, H, W = 16, 512, 32, 32
T = H * W              # 1024
NH = 8                 # heads
CH = C // NH           # 64
GROUPS = 32
GSIZE = C // GROUPS    # 16 channels per group
EPS = 1e-5
N_CORES = 8
BPC = B // N_CORES     # batches per core
CB = C // 128          # 4 channel blocks
NT = T // 512          # 2 column halves of 512
ST = T // 128          # 8 seq tiles of 128

DEBUG = bool(int(os.environ.get("KERNEL_DEBUG", "0")))


def _build():
    nc = bacc.Bacc(None, target_bir_lowering=False)

    x2 = nc.dram_tensor("x2", (BPC, C, T), F32, kind="ExternalInput")
    y2 = nc.dram_tensor("y2", (BPC, C, T), F32, kind="ExternalInput")
    wqt = nc.dram_tensor("wqt", (C, C), F32, kind="ExternalInput")
    wkt = nc.dram_tensor("wkt", (C, C), F32, kind="ExternalInput")
    wvt = nc.dram_tensor("wvt", (C, C), F32, kind="ExternalInput")
    wpt = nc.dram_tensor("wpt", (C, C), F32, kind="ExternalInput")
    bq_l = nc.dram_tensor("bq_l", (128, CB), F32, kind="ExternalInput")
    bk_l = nc.dram_tensor("bk_l", (128, CB), F32, kind="ExternalInput")
    bp_l = nc.dram_tensor("bp_l", (128, CB), F32, kind="ExternalInput")
    bv_bc = nc.dram_tensor("bv_bc", (128, NH, CH), F32, kind="ExternalInput")
    gnw_l = nc.dram_tensor("gnw_l", (128, CB), F32, kind="ExternalInput")
    gnb_l = nc.dram_tensor("gnb_l", (128, CB), F32, kind="ExternalInput")
    m1 = nc.dram_tensor("m1", (128, 128), F32, kind="ExternalInput")
    ones8 = nc.dram_tensor("ones8", (128, NH), BF16, kind="ExternalInput")
    out_d = nc.dram_tensor("out", (BPC, C, T), F32, kind="ExternalOutput")
    if DEBUG:
        dbg_a = nc.dram_tensor("dbg_a", (C, T), F32, kind="ExternalOutput")

    with tile.TileContext(nc) as tc:
        from contextlib import ExitStack
        with ExitStack() as ctx:
            consts = ctx.enter_context(tc.tile_pool(name="consts", bufs=1))
            px = ctx.enter_context(tc.tile_pool(name="px", bufs=2))
            py = ctx.enter_context(tc.tile_pool(name="py", bufs=1))
            pgn = ctx.enter_context(tc.tile_pool(name="pgn", bufs=2))
            pk = ctx.enter_context(tc.tile_pool(name="pk", bufs=4))
            pvt = ctx.enter_context(tc.tile_pool(name="pvt", bufs=2 * ST))
            pq = ctx.enter_context(tc.tile_pool(name="pq", bufs=3))
            pwts = ctx.enter_context(tc.tile_pool(name="pwts", bufs=10))
            pa = ctx.enter_context(tc.tile_pool(name="pa", bufs=1))
            pdn = ctx.enter_context(tc.tile_pool(name="pdn", bufs=1))
            pdn3 = ctx.enter_context(tc.tile_pool(name="pdn3", bufs=2))
            pst = ctx.enter_context(tc.tile_pool(name="pst", bufs=4))
            ps_mm = ctx.enter_context(tc.tile_pool(name="ps_mm", bufs=2, space="PSUM"))
            ps_s = ctx.enter_context(tc.tile_pool(name="ps_s", bufs=2, space="PSUM"))
            ps_a = ctx.enter_context(tc.tile_pool(name="ps_a", bufs=2, space="PSUM"))

            # --- constants (big weight DMAs emitted after batch-0 loads) ---
            wq_sb = consts.tile([128, CB, C], F32R, tag="wq")
            wk_sb = consts.tile([128, CB, C], F32R, tag="wk")
            wv_sb = consts.tile([128, CB, C], F32R, tag="wv")
            wp_sb = consts.tile([128, CB, C], F32R, tag="wp")

            def emit_weight_loads():
                nc.sync.dma_start(out=wk_sb, in_=wkt.rearrange("(kb p) o -> p kb o", p=128).bitcast(F32R))
                nc.sync.dma_start(out=wv_sb, in_=wvt.rearrange("(kb p) o -> p kb o", p=128).bitcast(F32R))
                nc.sync.dma_start(out=wq_sb, in_=wqt.rearrange("(kb p) o -> p kb o", p=128).bitcast(F32R))
                nc.sync.dma_start(out=wp_sb, in_=wpt.rearrange("(kb p) o -> p kb o", p=128).bitcast(F32R))

            m1_sb = consts.tile([128, 128], F32, tag="m1")
            bq_sb = consts.tile([128, CB], F32, tag="bq")
            bk_sb = consts.tile([128, CB], F32, tag="bk")
            bp_sb = consts.tile([128, CB], F32, tag="bp")
            bv_sb = consts.tile([128, NH, CH], F32, tag="bv")
            gnw_sb = consts.tile([128, CB], F32, tag="gnw")
            gnb_sb = consts.tile([128, CB], F32, tag="gnb")
            ones_sb = consts.tile([128, NH], BF16, tag="ones")
            magic_sb = consts.tile([128, CB], I32, tag="magic")
            nc.vector.memset(magic_sb, 0x5f3759df)
            warm = consts.tile([1, 1], F32, tag="warm")
            nc.vector.memset(warm, 0.0)
            nc.scalar.activation(out=warm, in_=warm, func=AF.Exp)

            def emit_small_consts():
                nc.sync.dma_start(out=m1_sb, in_=m1[:, :])
                nc.sync.dma_start(out=gnw_sb, in_=gnw_l[:, :])
                nc.sync.dma_start(out=gnb_sb, in_=gnb_l[:, :])
                nc.sync.dma_start(out=bk_sb, in_=bk_l[:, :])
                nc.sync.dma_start(out=bq_sb, in_=bq_l[:, :])
                nc.sync.dma_start(out=bv_sb, in_=bv_bc[:, :, :])
                nc.sync.dma_start(out=bp_sb, in_=bp_l[:, :])
                nc.sync.dma_start(out=ones_sb, in_=ones8[:, :])

            def groupnorm(src_sb, dst_fn, fast_apply=False, act_stats=False):
                """src_sb: [128, CB, T] f32. dst_fn(cb, th)->AP (f32r out)."""
                mv = pst.tile([128, CB, 2], F32, tag="mv")
                if act_stats:
                    # stats on the (idle at startup) ACT engine: mean via
                    # Copy-accum, E[x^2] via Square-accum; scale folds /T in.
                    part = pst.tile([128, CB, 2, 2], F32, tag="part")
                    for cb in range(CB):
                        for c2 in range(2):
                            trash = pwts.tile([128, 512], BF16, tag="wts", name="trash")
                            nc.scalar.activation(
                                out=trash, in_=src_sb[:, cb, c2 * 512:(c2 + 1) * 512],
                                func=AF.Copy, scale=1.0 / T,
                                accum_out=part[:, cb, 0, c2:c2 + 1])
                            trash2 = pwts.tile([128, 512], BF16, tag="wts", name="trash2")
                            nc.scalar.activation(
                                out=trash2, in_=src_sb[:, cb, c2 * 512:(c2 + 1) * 512],
                                func=AF.Square, scale=1.0 / np.sqrt(T),
                                accum_out=part[:, cb, 1, c2:c2 + 1])
                    # mv[:, cb, 0] = mean, mv[:, cb, 1] = E[x^2]
                    nc.vector.tensor_tensor(
                        out=mv.rearrange("p a b -> p (a b)"),
                        in0=part[:, :, :, 0].rearrange("p a b -> p (a b)"),
                        in1=part[:, :, :, 1].rearrange("p a b -> p (a b)"), op=OP.add)
                else:
                    stats6 = pst.tile([128, 2, 6], F32, tag="stats6")
                    for cb in range(CB):
                        for c2 in range(2):
                            nc.vector.bn_stats(
                                out=stats6[:, c2, :],
                                in_=src_sb[:, cb, c2 * 512:(c2 + 1) * 512])
                        nc.vector.bn_aggr(out=mv[:, cb, :], in_=stats6)
                    # m2 slot in-place: mv[:,:,1] = var + mean^2
                    musq = pst.tile([128, 4], F32, tag="musq")
                    nc.vector.tensor_tensor(out=musq, in0=mv[:, :, 0], in1=mv[:, :, 0], op=OP.mult)
                    nc.vector.tensor_tensor(out=mv[:, :, 1], in0=musq, in1=mv[:, :, 1], op=OP.add)
                psg = ps_mm.tile([128, 8], F32, tag="mm")
                nc.tensor.matmul(psg, m1_sb, mv.rearrange("p a b -> p (a b)"), start=True, stop=True)
                gsb = pst.tile([128, 8], F32, tag="gsb")
                nc.vector.tensor_copy(gsb, psg)  # m1 carries 1/GSIZE; cols interleaved (mean, m2)
                # var + eps = (m2 + eps) - mean^2, fused
                tmp4 = pst.tile([128, 4], F32, tag="tmp4")
                nc.vector.tensor_tensor(out=tmp4, in0=gsb[:, 0::2], in1=gsb[:, 0::2], op=OP.mult)
                vv = pst.tile([128, 4], F32, tag="vv")
                nc.vector.scalar_tensor_tensor(
                    out=vv, in0=gsb[:, 1::2], scalar=EPS, in1=tmp4,
                    op0=OP.add, op1=OP.subtract)
                # rstd = rsqrt(vv): quake seed + 2 Newton steps (3 fused ops each)
                bsh = pst.tile([128, 4], I32, tag="bsh")
                nc.vector.tensor_scalar(
                    out=bsh, in0=vv.bitcast(I32), scalar1=1, scalar2=None,
                    op0=OP.logical_shift_right)
                nc.vector.tensor_tensor(out=tmp4.bitcast(I32), in0=magic_sb, in1=bsh, op=OP.subtract)
                nrt = pst.tile([128, 4], F32, tag="nrt")
                for _ in range(2):
                    nc.vector.tensor_tensor(out=nrt, in0=tmp4, in1=tmp4, op=OP.mult)
                    nc.vector.scalar_tensor_tensor(
                        out=nrt, in0=nrt, scalar=-0.5, in1=vv, op0=OP.mult, op1=OP.mult)
                    nc.vector.scalar_tensor_tensor(
                        out=tmp4, in0=nrt, scalar=1.5, in1=tmp4, op0=OP.add, op1=OP.mult)
                ab = pst.tile([128, 8], F32, tag="ab")
                nc.vector.tensor_tensor(out=ab[:, 0:4], in0=tmp4, in1=gnw_sb, op=OP.mult)
                tmp4b = pst.tile([128, 4], F32, tag="tmp4b")
                nc.vector.tensor_tensor(out=tmp4b, in0=gsb[:, 0::2], in1=ab[:, 0:4], op=OP.mult)
                nc.vector.tensor_tensor(out=ab[:, 4:8], in0=gnb_sb, in1=tmp4b, op=OP.subtract)
                for th in range(NT):
                    for cb in range(CB):
                        eng = nc.gpsimd if (cb % 2 == 1 and not fast_apply) else nc.vector
                        eng.tensor_scalar(
                            out=dst_fn(cb, th), in0=src_sb[:, cb, th * 512:(th + 1) * 512],
                            scalar1=ab[:, cb:cb + 1], scalar2=ab[:, 4 + cb:5 + cb],
                            op0=OP.mult, op1=OP.add)

            def emit_input_loads(b):
                y_sb = py.tile([128, CB, T], F32, tag="y")
                for cb in range(CB):
                    nc.sync.dma_start(
                        out=y_sb[:, cb, :],
                        in_=y2[b].rearrange("(cb p) t -> p cb t", p=128)[:, cb, :])
                x_sb = px.tile([128, CB, T], F32, tag="x")
                for cb in range(CB):
                    nc.sync.dma_start(
                        out=x_sb[:, cb, :],
                        in_=x2[b].rearrange("(cb p) t -> p cb t", p=128)[:, cb, :])
                return x_sb, y_sb

            def emit_gn_compute(x_sb, y_sb, fast_apply=False, act_stats=False):
                gny = pgn.tile([128, CB, T], F32R, tag="gn")
                groupnorm(y_sb, lambda cb, th: gny[:, cb, th * 512:(th + 1) * 512], fast_apply)
                gnx = pgn.tile([128, CB, T], F32R, tag="gn")
                groupnorm(x_sb, lambda cb, th: gnx[:, cb, th * 512:(th + 1) * 512], fast_apply,
                          act_stats=act_stats)
                return gnx, gny

            def emit_gn_stage(b, fast_apply=False):
                x_sb, y_sb = emit_input_loads(b)
                return emit_gn_compute(x_sb, y_sb, fast_apply)

            def q_proj(ob, gnx):
                qp0 = pq.tile([128, T], F32R, tag="qpad")
                qp1 = pq.tile([128, T], F32R, tag="qpad")
                nc.gpsimd.memset(qp0[64:128, :].bitcast(F32), 0.0)
                nc.gpsimd.memset(qp1[0:64, :].bitcast(F32), 0.0)
                for th in range(NT):
                    psq = ps_mm.tile([128, 512], F32, tag="mm")
                    for kb in range(CB):
                        nc.tensor.matmul(
                            psq,
                            wq_sb[:, kb, ob * 128:(ob + 1) * 128],
                            gnx[:, kb, th * 512:(th + 1) * 512],
                            start=(kb == 0), stop=(kb == CB - 1))
                    nc.vector.tensor_scalar(
                        out=qp0[0:64, th * 512:(th + 1) * 512],
                        in0=psq[0:64, :], scalar1=bq_sb[0:64, ob:ob + 1],
                        scalar2=None, op0=OP.add)
                    nc.vector.tensor_scalar(
                        out=qp1[64:128, th * 512:(th + 1) * 512],
                        in0=psq[64:128, :], scalar1=bq_sb[64:128, ob:ob + 1],
                        scalar2=None, op0=OP.add)
                return qp0, qp1

            def k_proj(ob, gny):
                k_ob = pk.tile([128, T], F32R, tag="k", name="k_ob")
                for th in range(NT):
                    psk = ps_mm.tile([128, 512], F32, tag="mm")
                    for kb in range(CB):
                        nc.tensor.matmul(
                            psk,
                            wk_sb[:, kb, ob * 128:(ob + 1) * 128],
                            gny[:, kb, th * 512:(th + 1) * 512],
                            start=(kb == 0), stop=(kb == CB - 1))
                    nc.vector.tensor_scalar(
                        out=k_ob[:, th * 512:(th + 1) * 512],
                        in0=psk, scalar1=bk_sb[:, ob:ob + 1], scalar2=None, op0=OP.add)
                return k_ob

            def vt_proj(gny):
                vt_tiles = []
                for tt in range(ST):
                    psv = ps_mm.tile([128, 512], F32, tag="mm")
                    for kb in range(CB):
                        nc.tensor.matmul(
                            psv,
                            gny[:, kb, tt * 128:(tt + 1) * 128],
                            wv_sb[:, kb, :],
                            start=(kb == 0), stop=(kb == CB - 1))
                    vt = pvt.tile([128, NH, CH + 1], BF16, tag="vt", name="vt")
                    nc.vector.tensor_tensor(
                        out=vt[:, :, 0:CH],
                        in0=psv.rearrange("p (h c) -> p h c", h=NH),
                        in1=bv_sb, op=OP.add)
                    nc.vector.tensor_copy(vt[:, :, CH:CH + 1], ones_sb.rearrange("p (h o) -> p h o", o=1))
                    vt_tiles.append(vt)
                return vt_tiles

            def emit_prep_stage(bctx):
                """k for all obs + vT — the pre-attention PE work."""
                gnx, gny = bctx["gn"]
                bctx["k"] = [k_proj(ob, gny) for ob in range(CB)]
                bctx["vt"] = vt_proj(gny)

            def attention_head(bctx, ob, hh, qp, a_sb):
                h = 2 * ob + hh
                k_ob = bctx["k"][ob]
                vt_tiles = bctx["vt"]
                psa = [ps_a.tile([CH + 1, 512], F32, tag=f"av{i}", name=f"psa{i}") for i in range(NT)]
                for st in range(ST):
                    for th in range(NT):
                        pss = ps_s.tile([128, 512], F32, tag="sc")
                        nc.tensor.matmul(
                            pss,
                            k_ob[:, st * 128:(st + 1) * 128],
                            qp[:, th * 512:(th + 1) * 512],
                            start=True, stop=True)
                        wts = pwts.tile([128, 512], BF16, tag="wts")
                        nc.scalar.activation(out=wts, in_=pss, func=AF.Exp)
                        nc.tensor.matmul(
                            psa[th],
                            vt_tiles[st][:, h, :],
                            wts,
                            start=(st == 0), stop=(st == ST - 1))
                finish_head(ob, hh, psa, a_sb)

            def finish_head(ob, hh, psa, a_sb):
                # softmax denominator -> reciprocal -> broadcast -> normalize
                dcp = pdn3.tile([CH + 1, T], F32, tag="dn3")
                r0 = pdn.tile([1, T], F32, tag="r0")
                for th in range(NT):
                    nc.vector.tensor_copy(
                        dcp[CH:CH + 1, th * 512:(th + 1) * 512],
                        psa[th][CH:CH + 1, :])
                nc.sync.dma_start(out=r0, in_=dcp[CH:CH + 1, :])
                r0r = pdn3.tile([1, T], F32, tag="dn3")
                nc.vector.reciprocal_approx_fast(out=r0r, in_=r0)
                rbc = pdn.tile([64, T], F32, tag="rbc")
                nc.gpsimd.partition_broadcast(rbc, r0r)
                if hh == 0:
                    for th in range(NT):
                        nc.vector.tensor_tensor(
                            out=a_sb[0:64, ob, th * 512:(th + 1) * 512],
                            in0=psa[th][0:CH, :],
                            in1=rbc[:, th * 512:(th + 1) * 512], op=OP.mult)
                else:
                    a_tmp = pdn3.tile([64, T], F32R, tag="dn3")
                    for th in range(NT):
                        nc.vector.tensor_tensor(
                            out=a_tmp[:, th * 512:(th + 1) * 512],
                            in0=psa[th][0:CH, :],
                            in1=rbc[:, th * 512:(th + 1) * 512], op=OP.mult)
                    nc.sync.dma_start(out=a_sb[64:128, ob, :], in_=a_tmp)

            # ---------------- batch pipeline ----------------
            bctxs = [dict() for _ in range(BPC)]
            _xy0 = emit_input_loads(0)
            emit_small_consts()
            bctxs[0]["gn"] = emit_gn_compute(*_xy0)
            emit_weight_loads()

            for b in range(BPC):
                bctx = bctxs[b]
                gnx, gny = bctx["gn"]
                a_sb = pa.tile([128, CB, T], F32R, tag="a")
                bctx.setdefault("k", [])
                qps = bctx.pop("qps", {})
                for ob in range(CB):
                    if ob < CB - 1:
                        if ob >= len(bctx["k"]):
                            bctx["k"].append(k_proj(ob, gny))
                        if ob not in qps:
                            qps[ob] = q_proj(ob, gnx)
                    if ob == 0 and "vt" not in bctx:
                        bctx["vt"] = vt_proj(gny)
                    if ob == CB - 2:
                        # emit last block's projections early: frees gnx/gny a
                        # phase sooner so the next batch's prep can overlap
                        bctx["k"].append(k_proj(CB - 1, gny))
                        qps[CB - 1] = q_proj(CB - 1, gnx)
                    qp0, qp1 = qps.pop(ob)
                    for hh in range(2):
                        attention_head(bctx, ob, hh, (qp0, qp1)[hh], a_sb)
                    if ob == 0 and b + 1 < BPC:
                        bctxs[b + 1]["gn"] = emit_gn_stage(b + 1)

                if DEBUG and b == 0:
                    nc.sync.dma_start(out=dbg_a.rearrange("(cb p) t -> p cb t", p=128), in_=a_sb.bitcast(F32))

                # next batch's first projections jump the queue ahead of the
                # output projection so its attention starts at the boundary
                if b + 1 < BPC:
                    nb = bctxs[b + 1]
                    ngnx, ngny = nb["gn"]
                    nb["k"] = [k_proj(0, ngny), k_proj(1, ngny)]
                    nb["qps"] = {0: q_proj(0, ngnx)}
                    nb["vt"] = vt_proj(ngny)

                # --- output projection + bias + residual (x reloaded fresh) ---
                xr = px.tile([128, CB, T], F32, tag="x")
                nc.sync.dma_start(out=xr, in_=x2[b].rearrange("(cb p) t -> p cb t", p=128))
                for ob in range(CB):
                    for th in range(NT):
                        psh = ps_mm.tile([128, 512], F32, tag="mm")
                        for kb in range(CB):
                            nc.tensor.matmul(
                                psh,
                                wp_sb[:, kb, ob * 128:(ob + 1) * 128],
                                a_sb[:, kb, th * 512:(th + 1) * 512],
                                start=(kb == 0), stop=(kb == CB - 1))
                        nc.vector.scalar_tensor_tensor(
                            out=xr[:, ob, th * 512:(th + 1) * 512],
                            in0=psh, scalar=bp_sb[:, ob:ob + 1],
                            in1=xr[:, ob, th * 512:(th + 1) * 512],
                            op0=OP.add, op1=OP.add)
                    nc.sync.dma_start(
                        out=out_d[b].rearrange("(cb p) t -> p cb t", p=128)[:, ob, :],
                        in_=xr[:, ob, :])

    nc.finalize()
    return nc


_NC = None


def _get_nc():
    global _NC
    if _NC is None:
        _NC = _build()
    return _NC


def _prep_inputs(x, y, gn_w, gn_b, Wq, bq, Wkv, bkv, Wp, bp):
    scale = CH ** -0.25
    # reference splits k/v per head: kvh[:, h, :ch] / kvh[:, h, ch:] after
    # reshape to [b, NH, 2*ch, T] -> k_h = Wkv rows [h*128, h*128+64)
    idx_k = np.concatenate([np.arange(h * 2 * CH, h * 2 * CH + CH) for h in range(NH)])
    idx_v = np.concatenate([np.arange(h * 2 * CH + CH, (h + 1) * 2 * CH) for h in range(NH)])
    wqt = np.ascontiguousarray((Wq * scale).T)          # [C, C] (c_in, o)
    wkt = np.ascontiguousarray((Wkv[idx_k] * scale).T)
    wvt = np.ascontiguousarray(Wkv[idx_v].T)
    wpt = np.ascontiguousarray(Wp.T)
    bq_s = bq * scale
    bk_s = bkv[idx_k] * scale
    bv = bkv[idx_v]

    def part_layout(v):  # [C] -> [128, CB]: v[cb*128+p]
        return np.ascontiguousarray(v.reshape(CB, 128).T)

    bq_l = part_layout(bq_s)
    bk_l = part_layout(bk_s)
    bp_l = part_layout(bp)
    gnw_l = part_layout(gn_w)
    gnb_l = part_layout(gn_b)
    bv_bc = np.broadcast_to(bv.reshape(1, NH, CH), (128, NH, CH)).copy()
    m1 = np.zeros((128, 128), np.float32)
    for g in range(128 // GSIZE):
        m1[g * GSIZE:(g + 1) * GSIZE, g * GSIZE:(g + 1) * GSIZE] = 1.0 / GSIZE
    import ml_dtypes
    ones8 = np.ones((128, NH), ml_dtypes.bfloat16)

    xf = x.reshape(B, C, T)
    yf = y.reshape(B, C, T)

    shared = {
        "wqt": wqt, "wkt": wkt, "wvt": wvt, "wpt": wpt,
        "bq_l": bq_l, "bk_l": bk_l, "bp_l": bp_l, "bv_bc": bv_bc,
        "gnw_l": gnw_l, "gnb_l": gnb_l, "m1": m1, "ones8": ones8,
    }
    in_maps = []
    for i in range(N_CORES):
        m = dict(shared)
        m["x2"] = np.ascontiguousarray(xf[i * BPC:(i + 1) * BPC])
        m["y2"] = np.ascontiguousarray(yf[i * BPC:(i + 1) * BPC])
        in_maps.append(m)
    return in_maps


def kernel(x, y, gn_w, gn_b, Wq, bq, Wkv, bkv, Wp, bp):
    args = [np.asarray(a, dtype=np.float32) for a in
            (x, y, gn_w, gn_b, Wq, bq, Wkv, bkv, Wp, bp)]
    in_maps = _prep_inputs(*args)
    nc = _get_nc()
    res = run_bass_kernel_spmd(nc, in_maps, core_ids=list(range(N_CORES)))
    out = np.empty((B, C, T), np.float32)
    for i in range(N_CORES):
        out[i * BPC:(i + 1) * BPC] = res.results[i]["out"]
    return out.reshape(B, C, H, W)

